# revision 4
# baseline (speedup 1.0000x reference)
"""Trainium2 Bass kernel for nn_MeshLoss.

The reference loss is:
    loss = mean((network_mesh - fem_mesh)^2)
         + 0.1 * sum_{dx,dy,dz} sum_spatial(mean_{B,C}(diff^2))
The chamfer/KNN block in the reference is dead code (its results are unused),
and `pc` does not influence the output, so the kernel computes only the two
reduction terms.

Sharding (8 cores): pred is viewed as 12*32 = 384 (bc, x) planes of [32, 32];
the 12*31 = 372 planes with x < 31 are regularization bases, 46-47 per core.
On the host each (plane, y<31) pair becomes a 4-row unit [base row, y+1 row,
x+1-plane row, z+1-shifted row]; a core's 48*31 units (zero-padded to 1536)
span all 128 SBUF partitions, so ALL three regularization differences are one
elementwise subtract of rows 1:4 against a stride-0 broadcast of row 0, with
the y/z ":-1" bounds expressed as strided access patterns.

Each core loads one unified [128, 2304] bf16 tile as two contiguous HBM loads
on separate HWDGE rings: ld_a = the 1536 unit columns (ACT ring), ld_b = the
net[384] ++ fem[384] columns (SP ring).  Every instruction then depends on at
most one DMA semaphore (walrus rejects >2 sync commands).

Compute: Vector does the fem subtract, a fused square+accumulate
(tensor_tensor_reduce) on the fem diff, and the single reg subtract; the
Scalar/ACT engine squares the 1116 reg diff columns (activation Square with
accum_out) in parallel.  Two output DMAs, each on the producing engine's own
ring: fem partial sums [128,1] (SP, waits Vector) and reg partial sums
[128,1] (ACT, program order).  The host sums the 8 cores' outputs and
applies the 1/N and 0.1/12 weights.

BIR post-processing before compile: the entry barrier is stripped, the whole
Tile tail (drains/barriers/semaphore clear) is dropped (the runtime epilogue
rendezvouses the engines and clears every semaphore anyway), the PE engine's
instructions (register init + branches only -- it does no work) are deleted
so the NEFF carries no PE program and the runtime entry rendezvous does not
wait ~2.5us for the PE array init, and the input-load DMA triggers are
hoisted to the head of the program so the HBM->SBUF transfers start as soon
as the runtime preamble ends.
"""

import numpy as np

B, C, X, Y, Z = 4, 3, 32, 32, 32
N_CORES = 8
FEM_TOTAL = B * C * X * Y * Z          # 393216
REG_PLANES = B * C * (X - 1)           # 372 valid base planes
PLANES_PC = 48                         # plane slots per core (8*48 = 384)
UNITS_PC = PLANES_PC * (Y - 1)         # 1488 (plane, y) units per core
KU = 12                                # units per partition (128*12 = 1536)
FEM_P, FEM_F = 128, FEM_TOTAL // N_CORES // 128   # [128, 384] per core
UW = KU * 4 * Z                        # 1536 unit columns
NF = 2 * FEM_F                         # 768 net+fem columns
LW = UW + NF                           # 2304 total columns
DREG = KU * 3 * 31                     # 1116 reg diff cols

_PROGRAM = None
_HOOK_PATCHED = False
# Bump whenever the BIR post-edit logic changes: the neuron compile cache
# keys on the HLO (which embeds the *unpatched* BIR), so a patch-logic change
# must perturb the program to force a recompile.
_BIR_REV = 21


def _strip_pe(bir_json):
    """Delete every PE-engine instruction (register init + branches only --
    the kernel does no matmul).  With no PE instructions the NEFF carries no
    PE program, so the runtime entry rendezvous doesn't wait ~2.5us for the
    PE array init."""
    import json

    j = json.loads(bir_json)
    for f in j.get("functions", []):
        for bb in f.get("blocks", []):
            kept = []
            for i in bb.get("instructions", []):
                if i.get("engine") == "PE":
                    op = i.get("opcode")
                    assert op in ("RegisterMove", "UnconditionalBranch",
                                  "Drain", "EventSemaphore"), op
                else:
                    kept.append(i)
            bb["instructions"] = kept
    return json.dumps(j).encode()


def _strip_barriers_and_tail(bir_json):
    """Remove the Tile entry barrier (block 0) and the entire Tile tail
    block (drains, two barrier rounds, semaphore range-clear).  The runtime
    epilogue independently rendezvouses all engines and zeroes every
    semaphore before the next execution, so the Tile tail only adds
    critical-path time."""
    import json

    j = json.loads(bir_json)
    for f in j.get("functions", []):
        blocks = f.get("blocks", [])
        if not blocks:
            continue
        blocks[0]["instructions"] = [
            i for i in blocks[0].get("instructions", [])
            if i.get("opcode") not in ("Drain", "EventSemaphore")
        ]
        bl = blocks[-1]
        bl["instructions"] = [
            i for i in bl.get("instructions", [])
            if i.get("opcode") not in ("Drain", "EventSemaphore", "ISA")
        ]
    return json.dumps(j).encode()


def _hoist_input_dmas(bir_json, input_names=("ld_a", "ld_b")):
    """Move the input-load DMA triggers to the head of the first block so the
    HBM->SBUF transfers start as soon as the runtime preamble ends, ahead of
    the register-init instructions.  The triggers have no waits and their
    DMAHW semaphore updates don't interact with anything earlier, so
    ordering stays sound."""
    import json

    j = json.loads(bir_json)
    for f in j.get("functions", []):
        blocks = f.get("blocks", [])
        if not blocks:
            continue
        existing = {i.get("name") for bb in blocks for i in bb.get("instructions", [])}
        hoisted = []
        for bb in blocks:
            insts = bb.get("instructions", [])
            keep = []
            for i in insts:
                ins0 = (i.get("ins") or [{}])[0]
                if (i.get("opcode") == "DMACopy"
                        and not (i.get("sync_info") or {}).get("on_wait")
                        and ins0.get("memref") in input_names):
                    hoisted.append(i)
                else:
                    keep.append(i)
            bb["instructions"] = keep
        # Renumber so they sort before everything even if the backend orders
        # by instruction id rather than list position.
        for n, i in enumerate(hoisted):
            name = f"I-{n}"
            while name in existing:
                name += "h"
            existing.add(name)
            i["name"] = name
            i["debug"] = 1
        blocks[0]["instructions"] = hoisted + blocks[0]["instructions"]
    return json.dumps(j).encode()


def _patch_compile_hook():
    global _HOOK_PATCHED
    if _HOOK_PATCHED:
        return
    import concourse.bass2jax as b2j

    orig = b2j.compile_bir_kernel

    def patched(bir_json, tmpdir, neff_name="file.neff"):
        return orig(_hoist_input_dmas(_strip_barriers_and_tail(
            _strip_pe(bir_json))), tmpdir, neff_name=neff_name)

    b2j.compile_bir_kernel = patched
    _HOOK_PATCHED = True


def _build_program():
    import concourse.bass as bass
    import concourse.mybir as mybir
    from concourse import tile
    from contextlib import ExitStack

    f32 = mybir.dt.float32
    bf16 = mybir.dt.bfloat16
    SUB = mybir.AluOpType.subtract
    MULT = mybir.AluOpType.mult
    ADD = mybir.AluOpType.add
    SQUARE = mybir.ActivationFunctionType.Square

    nc = bass.Bass()
    nc.dram_tensor(f"patchrev{_BIR_REV}", [1, 1], f32)
    ld_a = nc.declare_dram_parameter("ld_a", [128, UW], bf16, isOutput=False)
    ld_b = nc.declare_dram_parameter("ld_b", [128, NF], bf16, isOutput=False)
    out_f = nc.declare_dram_parameter("out_f", [128, 1], f32, isOutput=True)
    out_r = nc.declare_dram_parameter("out_r", [128, 1], f32, isOutput=True)

    with tile.TileContext(nc) as tc, ExitStack() as ctx:
        pool = ctx.enter_context(tc.tile_pool(name="main", bufs=1))

        t_l = pool.tile([128, LW], bf16)
        nc.scalar.dma_start(out=t_l[:, 0:UW], in_=ld_a[:, :])
        nc.sync.dma_start(out=t_l[:, UW:LW], in_=ld_b[:, :])

        t_u = t_l[:, 0:UW].rearrange("p (k r z) -> p k r z", k=KU, r=4)
        t_d = pool.tile([128, DREG + FEM_F], bf16)
        t_sq = pool.tile([128, DREG], bf16)
        t_sqf = pool.tile([128, FEM_F], bf16)
        t_accf = pool.tile([128, 1], f32)
        t_accr = pool.tile([128, 1], f32)

        d_reg = t_d[:, 0:DREG].rearrange("p (k r z) -> p k r z", k=KU, r=3)
        d_f = t_d[:, DREG:DREG + FEM_F]

        # Vector: fem subtract + fused square-accumulate, then the single
        # reg subtract (rows 1:4 minus broadcast row 0).
        nc.vector.tensor_tensor(
            out=d_f, in0=t_l[:, UW:UW + FEM_F],
            in1=t_l[:, UW + FEM_F:LW], op=SUB)
        nc.vector.scalar_tensor_tensor(
            out=t_sqf[:], in0=d_f, scalar=1.0, in1=d_f,
            op0=MULT, op1=MULT, accum_out=t_accf[:])
        nc.vector.tensor_tensor(
            out=d_reg, in0=t_u[:, :, 1:4, 0:31],
            in1=t_u[:, :, 0:1, 0:31].broadcast_to([128, KU, 3, 31]), op=SUB)

        # Scalar/ACT squares the reg diffs while Vector works.
        nc.scalar.activation(
            out=t_sq[:], in_=t_d[:, 0:DREG], func=SQUARE,
            accum_out=t_accr[:])

        # Per-engine output DMAs (each has at most one sync wait).
        nc.sync.dma_start(out=out_f[:, :], in_=t_accf[:])
        nc.scalar.dma_start(out=out_r[:, :], in_=t_accr[:])

    return nc


def _shard_inputs(network_mesh, fem_mesh, pred):
    import ml_dtypes
    bf16 = ml_dtypes.bfloat16
    predf = np.asarray(pred, dtype=np.float32).reshape(B * C, X, Y, Z)
    pad = N_CORES * PLANES_PC
    base_p = np.zeros((pad, Y, Z), np.float32)
    nxt_p = np.zeros((pad, Y, Z), np.float32)
    base_p[:REG_PLANES] = predf[:, : X - 1].reshape(REG_PLANES, Y, Z)
    nxt_p[:REG_PLANES] = predf[:, 1:].reshape(REG_PLANES, Y, Z)
    zsh = np.zeros((pad, Y - 1, Z), np.float32)
    zsh[:, :, : Z - 1] = base_p[:, : Y - 1, 1:]
    # [384, 31, 4, 32]: per (plane, y): base, y+1, x+1-plane, z+1 rows
    u_all = np.stack(
        [base_p[:, : Y - 1], base_p[:, 1:], nxt_p[:, : Y - 1], zsh], axis=2
    )
    netf = np.asarray(network_mesh, dtype=np.float32).reshape(N_CORES, FEM_P, FEM_F)
    femf = np.asarray(fem_mesh, dtype=np.float32).reshape(N_CORES, FEM_P, FEM_F)
    maps = []
    for c in range(N_CORES):
        uc = u_all[PLANES_PC * c : PLANES_PC * (c + 1)].reshape(UNITS_PC, 4, Z)
        up = np.zeros((128 * KU, 4, Z), np.float32)
        up[:UNITS_PC] = uc
        maps.append({
            "ld_a": up.reshape(128, UW).astype(bf16),
            "ld_b": np.ascontiguousarray(
                np.concatenate([netf[c], femf[c]], axis=1)).astype(bf16),
        })
    return maps


def run_sharded(network_mesh, fem_mesh, pred, trace=False):
    """Compile+run on 8 cores; returns (loss_scalar, BassKernelResults)."""
    global _PROGRAM
    from concourse.bass_utils import run_bass_kernel_spmd

    _patch_compile_hook()
    if _PROGRAM is None:
        _PROGRAM = _build_program()
    in_maps = _shard_inputs(network_mesh, fem_mesh, pred)
    res = run_bass_kernel_spmd(_PROGRAM, in_maps, list(range(N_CORES)), trace=trace)
    fem_sum = 0.0
    reg_sum = 0.0
    for c in range(N_CORES):
        fem_sum += np.asarray(res.results[c]["out_f"], dtype=np.float64).sum()
        reg_sum += np.asarray(res.results[c]["out_r"], dtype=np.float64).sum()
    loss = fem_sum / FEM_TOTAL + 0.1 * (reg_sum / (B * C))
    return np.asarray(loss, dtype=np.float32), res


def kernel(network_mesh, pc, fem_mesh, pred):
    loss, _ = run_sharded(network_mesh, fem_mesh, pred, trace=False)
    return loss


# revision 7
# speedup vs baseline: 1.2652x; 1.2652x over previous
"""Trainium2 Bass kernel for nn_MeshLoss.

The reference loss is:
    loss = mean((network_mesh - fem_mesh)^2)
         + 0.1 * sum_{dx,dy,dz} sum_spatial(mean_{B,C}(diff^2))
The chamfer/KNN block in the reference is dead code (its results are unused),
and `pc` does not influence the output, so the kernel computes only the two
reduction terms.

Sharding (8 cores): pred is viewed as 12*32 = 384 (bc, x) planes of [32, 32];
the 12*31 = 372 planes with x < 31 are regularization bases, 46-47 per core.
On the host each (plane, y<31) pair becomes a 4-row unit [base row, y+1 row,
x+1-plane row, z+1-shifted row]; a core's 48*31 units (zero-padded to 1536)
span all 128 SBUF partitions, so ALL three regularization differences are one
elementwise subtract of rows 1:4 against a stride-0 broadcast of row 0, with
the y/z ":-1" bounds expressed as strided access patterns.

Each core loads one unified [128, 2304] bf16 tile as two contiguous HBM loads
on separate HWDGE rings: ld_a = the 1536 unit columns (ACT ring), ld_b = the
net[384] ++ fem[384] columns (SP ring).  Every instruction then depends on at
most one DMA semaphore (walrus rejects >2 sync commands).

Compute: Vector does the fem subtract, a fused square+accumulate
(tensor_tensor_reduce) on the fem diff, and the single reg subtract; the
Scalar/ACT engine squares the 1116 reg diff columns (activation Square with
accum_out) in parallel.  Two output DMAs, each on the producing engine's own
ring: fem partial sums [128,1] (SP, waits Vector) and reg partial sums
[128,1] (ACT, program order).  The host sums the 8 cores' outputs and
applies the 1/N and 0.1/12 weights.

BIR post-processing before compile: the entry barrier is stripped, the whole
Tile tail (drains/barriers/semaphore clear) is dropped (the runtime epilogue
rendezvouses the engines and clears every semaphore anyway), the PE engine's
instructions (register init + branches only -- it does no work) are deleted
so the NEFF carries no PE program and the runtime entry rendezvous does not
wait ~2.5us for the PE array init, and the input-load DMA triggers are
hoisted to the head of the program so the HBM->SBUF transfers start as soon
as the runtime preamble ends.
"""

import numpy as np

B, C, X, Y, Z = 4, 3, 32, 32, 32
N_CORES = 8
FEM_TOTAL = B * C * X * Y * Z          # 393216
REG_PLANES = B * C * (X - 1)           # 372 valid base planes
PLANES_PC = 48                         # plane slots per core (8*48 = 384)
UNITS_PC = PLANES_PC * (Y - 1)         # 1488 (plane, y) units per core
KU = 12                                # units per partition (128*12 = 1536)
FEM_P, FEM_F = 128, FEM_TOTAL // N_CORES // 128   # [128, 384] per core
UW = KU * 4 * Z                        # 1536 unit columns
NF = 2 * FEM_F                         # 768 net+fem columns
LW = UW + NF                           # 2304 total columns
DREG = KU * 3 * 31                     # 1116 reg diff cols

_PROGRAM = None
_HOOK_PATCHED = False
# Bump whenever the BIR post-edit logic changes: the neuron compile cache
# keys on the HLO (which embeds the *unpatched* BIR), so a patch-logic change
# must perturb the program to force a recompile.
_BIR_REV = 22


def _strip_pe(bir_json):
    """Delete every PE-engine instruction (register init + branches only --
    the kernel does no matmul).  With no PE instructions the NEFF carries no
    PE program, so the runtime entry rendezvous doesn't wait ~2.5us for the
    PE array init."""
    import json

    j = json.loads(bir_json)
    for f in j.get("functions", []):
        for bb in f.get("blocks", []):
            kept = []
            for i in bb.get("instructions", []):
                if i.get("engine") == "PE":
                    op = i.get("opcode")
                    assert op in ("RegisterMove", "UnconditionalBranch",
                                  "Drain", "EventSemaphore"), op
                else:
                    kept.append(i)
            bb["instructions"] = kept
    return json.dumps(j).encode()


def _fix_tail(bir_json):
    """Tail surgery.  (1) Strip the Tile entry barrier (block 0).  (2) In the
    end block: empty the multi-wait drain (walrus's >2-sync limit aside, its
    waits are covered by (3)), insert a Drain on the output DMA's engine that
    waits for the output DMAHW semaphore to reach 16 so NO engine reaches the
    runtime epilogue while the output write is in flight (the epilogue's
    semaphore-clear storm otherwise races the in-flight completion
    acknowledgments and stalls the semaphore bus for ~7us), keep the first
    barrier round with the Pool master counts dropped from 4 to 3 (PE is
    stripped), and delete the semaphore range-clear + second barrier round
    (the runtime epilogue re-zeroes every semaphore anyway)."""
    import json

    j = json.loads(bir_json)
    for f in j.get("functions", []):
        blocks = f.get("blocks", [])
        if not blocks:
            continue
        blocks[0]["instructions"] = [
            i for i in blocks[0].get("instructions", [])
            if i.get("opcode") not in ("Drain", "EventSemaphore")
        ]
        # Locate the output DMA (writes the "out" DRAM param).
        out_upd = out_eng = None
        for bb in blocks:
            for i in bb.get("instructions", []):
                if (i.get("opcode") == "DMACopy"
                        and (i.get("outs") or [{}])[0].get("memref") == "out"):
                    out_upd = (i["sync_info"]["on_update"] or [None])[0]
                    out_eng = i.get("engine")
        assert out_upd is not None and out_eng is not None
        bl = blocks[-1]
        insts = bl.get("instructions", [])
        # Drop the range-clear and the second barrier round after it.
        isa_idx = next((n for n, i in enumerate(insts)
                        if i.get("opcode") == "ISA"), len(insts))
        insts = insts[:isa_idx]
        for i in insts:
            si = i.get("sync_info") or {}
            if i.get("opcode") == "Drain" and len(si.get("on_wait") or []) > 1:
                si["on_wait"] = []
            if (i.get("opcode") == "EventSemaphore"
                    and i.get("engine") == "Pool"):
                for part in ("on_wait", "on_update"):
                    for e in si.get(part) or []:
                        if e.get("wait_value") == 4:
                            e["wait_value"] = 3
                        if e.get("update_value") == 4:
                            e["update_value"] = 3
        hold = {
            "debug": 1,
            "engine": out_eng,
            "ins": [],
            "name": "I-holdout",
            "opcode": "Drain",
            "outs": [],
            "sync_info": {
                "on_update": [],
                "on_wait": [{
                    "ant_name": out_upd["ant_name"],
                    "id": out_upd["id"],
                    "sync_type": "semaphore",
                    "wait_mode": "sem-ge-imm",
                    "wait_value": 16,
                }],
            },
        }
        bl["instructions"] = [hold] + insts
    return json.dumps(j).encode()


def _hoist_input_dmas(bir_json, input_names=("ld_a", "ld_b")):
    """Move the input-load DMA triggers to the head of the first block so the
    HBM->SBUF transfers start as soon as the runtime preamble ends, ahead of
    the register-init instructions.  The triggers have no waits and their
    DMAHW semaphore updates don't interact with anything earlier, so
    ordering stays sound."""
    import json

    j = json.loads(bir_json)
    for f in j.get("functions", []):
        blocks = f.get("blocks", [])
        if not blocks:
            continue
        existing = {i.get("name") for bb in blocks for i in bb.get("instructions", [])}
        hoisted = []
        for bb in blocks:
            insts = bb.get("instructions", [])
            keep = []
            for i in insts:
                ins0 = (i.get("ins") or [{}])[0]
                if (i.get("opcode") == "DMACopy"
                        and not (i.get("sync_info") or {}).get("on_wait")
                        and ins0.get("memref") in input_names):
                    hoisted.append(i)
                else:
                    keep.append(i)
            bb["instructions"] = keep
        # Renumber so they sort before everything even if the backend orders
        # by instruction id rather than list position.
        for n, i in enumerate(hoisted):
            name = f"I-{n}"
            while name in existing:
                name += "h"
            existing.add(name)
            i["name"] = name
            i["debug"] = 1
        blocks[0]["instructions"] = hoisted + blocks[0]["instructions"]
    return json.dumps(j).encode()


def _patch_compile_hook():
    global _HOOK_PATCHED
    if _HOOK_PATCHED:
        return
    import concourse.bass2jax as b2j

    orig = b2j.compile_bir_kernel

    def patched(bir_json, tmpdir, neff_name="file.neff"):
        return orig(_hoist_input_dmas(_fix_tail(
            _strip_pe(bir_json))), tmpdir, neff_name=neff_name)

    b2j.compile_bir_kernel = patched
    _HOOK_PATCHED = True


def _build_program():
    import concourse.bass as bass
    import concourse.mybir as mybir
    from concourse import tile
    from contextlib import ExitStack

    f32 = mybir.dt.float32
    bf16 = mybir.dt.bfloat16
    SUB = mybir.AluOpType.subtract
    MULT = mybir.AluOpType.mult
    ADD = mybir.AluOpType.add
    SQUARE = mybir.ActivationFunctionType.Square

    nc = bass.Bass()
    nc.dram_tensor(f"patchrev{_BIR_REV}", [1, 1], f32)
    ld_a = nc.declare_dram_parameter("ld_a", [128, UW], bf16, isOutput=False)
    ld_b = nc.declare_dram_parameter("ld_b", [128, NF], bf16, isOutput=False)
    out = nc.declare_dram_parameter("out", [128, 2], f32, isOutput=True)

    with tile.TileContext(nc) as tc, ExitStack() as ctx:
        pool = ctx.enter_context(tc.tile_pool(name="main", bufs=1))

        t_l = pool.tile([128, LW], bf16)
        nc.scalar.dma_start(out=t_l[:, 0:UW], in_=ld_a[:, :])
        nc.gpsimd.dma_start(out=t_l[:, UW:LW], in_=ld_b[:, :])

        t_u = t_l[:, 0:UW].rearrange("p (k r z) -> p k r z", k=KU, r=4)
        t_d = pool.tile([128, DREG + FEM_F], bf16)
        t_sq = pool.tile([128, DREG], bf16)
        t_sqf = pool.tile([128, FEM_F], bf16)
        t_acc = pool.tile([128, 2], f32)

        d_reg = t_d[:, 0:DREG].rearrange("p (k r z) -> p k r z", k=KU, r=3)
        d_f = t_d[:, DREG:DREG + FEM_F]

        # Vector: fem subtract + fused square-accumulate, then the single
        # reg subtract (rows 1:4 minus broadcast row 0).
        nc.vector.tensor_tensor(
            out=d_f, in0=t_l[:, UW:UW + FEM_F],
            in1=t_l[:, UW + FEM_F:LW], op=SUB)
        nc.vector.scalar_tensor_tensor(
            out=t_sqf[:], in0=d_f, scalar=1.0, in1=d_f,
            op0=MULT, op1=MULT, accum_out=t_acc[:, 0:1])
        nc.vector.tensor_tensor(
            out=d_reg, in0=t_u[:, :, 1:4, 0:31],
            in1=t_u[:, :, 0:1, 0:31].broadcast_to([128, KU, 3, 31]), op=SUB)

        # Scalar/ACT squares the reg diffs while Vector works.
        nc.scalar.activation(
            out=t_sq[:], in_=t_d[:, 0:DREG], func=SQUARE,
            accum_out=t_acc[:, 1:2])

        # Single output DMA on the ACT ring: column 0 (fem) is guarded by
        # the Vector semaphore wait; column 1 (reg) by ACT program order.
        nc.scalar.dma_start(out=out[:, :], in_=t_acc[:])

    return nc


def _shard_inputs(network_mesh, fem_mesh, pred):
    import ml_dtypes
    bf16 = ml_dtypes.bfloat16
    predf = np.asarray(pred, dtype=np.float32).reshape(B * C, X, Y, Z)
    pad = N_CORES * PLANES_PC
    base_p = np.zeros((pad, Y, Z), np.float32)
    nxt_p = np.zeros((pad, Y, Z), np.float32)
    base_p[:REG_PLANES] = predf[:, : X - 1].reshape(REG_PLANES, Y, Z)
    nxt_p[:REG_PLANES] = predf[:, 1:].reshape(REG_PLANES, Y, Z)
    zsh = np.zeros((pad, Y - 1, Z), np.float32)
    zsh[:, :, : Z - 1] = base_p[:, : Y - 1, 1:]
    # [384, 31, 4, 32]: per (plane, y): base, y+1, x+1-plane, z+1 rows
    u_all = np.stack(
        [base_p[:, : Y - 1], base_p[:, 1:], nxt_p[:, : Y - 1], zsh], axis=2
    )
    netf = np.asarray(network_mesh, dtype=np.float32).reshape(N_CORES, FEM_P, FEM_F)
    femf = np.asarray(fem_mesh, dtype=np.float32).reshape(N_CORES, FEM_P, FEM_F)
    maps = []
    for c in range(N_CORES):
        uc = u_all[PLANES_PC * c : PLANES_PC * (c + 1)].reshape(UNITS_PC, 4, Z)
        up = np.zeros((128 * KU, 4, Z), np.float32)
        up[:UNITS_PC] = uc
        maps.append({
            "ld_a": up.reshape(128, UW).astype(bf16),
            "ld_b": np.ascontiguousarray(
                np.concatenate([netf[c], femf[c]], axis=1)).astype(bf16),
        })
    return maps


def run_sharded(network_mesh, fem_mesh, pred, trace=False):
    """Compile+run on 8 cores; returns (loss_scalar, BassKernelResults)."""
    global _PROGRAM
    from concourse.bass_utils import run_bass_kernel_spmd

    _patch_compile_hook()
    if _PROGRAM is None:
        _PROGRAM = _build_program()
    in_maps = _shard_inputs(network_mesh, fem_mesh, pred)
    res = run_bass_kernel_spmd(_PROGRAM, in_maps, list(range(N_CORES)), trace=trace)
    fem_sum = 0.0
    reg_sum = 0.0
    for c in range(N_CORES):
        o = np.asarray(res.results[c]["out"], dtype=np.float64)
        fem_sum += o[:, 0].sum()
        reg_sum += o[:, 1].sum()
    loss = fem_sum / FEM_TOTAL + 0.1 * (reg_sum / (B * C))
    return np.asarray(loss, dtype=np.float32), res


def kernel(network_mesh, pc, fem_mesh, pred):
    loss, _ = run_sharded(network_mesh, fem_mesh, pred, trace=False)
    return loss


# revision 8
# speedup vs baseline: 1.2883x; 1.0182x over previous
"""Trainium2 Bass kernel for nn_MeshLoss.

The reference loss is:
    loss = mean((network_mesh - fem_mesh)^2)
         + 0.1 * sum_{dx,dy,dz} sum_spatial(mean_{B,C}(diff^2))
The chamfer/KNN block in the reference is dead code (its results are unused),
and `pc` does not influence the output, so the kernel computes only the two
reduction terms.

Sharding (8 cores): pred is viewed as 12*32 = 384 (bc, x) planes of [32, 32];
the 12*31 = 372 planes with x < 31 are regularization bases, 46-47 per core.
On the host each (plane, y<31) pair becomes a 4-row unit [base row, y+1 row,
x+1-plane row, z+1-shifted row]; a core's 48*31 units (zero-padded to 1536)
span all 128 SBUF partitions, so ALL three regularization differences are one
elementwise subtract of rows 1:4 against a stride-0 broadcast of row 0, with
the y/z ":-1" bounds expressed as strided access patterns.

Each core loads one unified [128, 2304] bf16 tile as two contiguous HBM loads
on separate HWDGE rings: ld_a = the 1536 unit columns (ACT ring), ld_b = the
net[384] ++ fem[384] columns (SP ring).  Every instruction then depends on at
most one DMA semaphore (walrus rejects >2 sync commands).

Compute: Vector does the fem subtract, a fused square+accumulate
(tensor_tensor_reduce) on the fem diff, and the single reg subtract; the
Scalar/ACT engine squares the 1116 reg diff columns (activation Square with
accum_out) in parallel.  Two output DMAs, each on the producing engine's own
ring: fem partial sums [128,1] (SP, waits Vector) and reg partial sums
[128,1] (ACT, program order).  The host sums the 8 cores' outputs and
applies the 1/N and 0.1/12 weights.

BIR post-processing before compile: the entry barrier is stripped, the whole
Tile tail (drains/barriers/semaphore clear) is dropped (the runtime epilogue
rendezvouses the engines and clears every semaphore anyway), the PE engine's
instructions (register init + branches only -- it does no work) are deleted
so the NEFF carries no PE program and the runtime entry rendezvous does not
wait ~2.5us for the PE array init, and the input-load DMA triggers are
hoisted to the head of the program so the HBM->SBUF transfers start as soon
as the runtime preamble ends.
"""

import numpy as np

B, C, X, Y, Z = 4, 3, 32, 32, 32
N_CORES = 8
FEM_TOTAL = B * C * X * Y * Z          # 393216
REG_PLANES = B * C * (X - 1)           # 372 valid base planes
PLANES_PC = 48                         # plane slots per core (8*48 = 384)
UNITS_PC = PLANES_PC * (Y - 1)         # 1488 (plane, y) units per core
KU = 12                                # units per partition (128*12 = 1536)
FEM_P, FEM_F = 128, FEM_TOTAL // N_CORES // 128   # [128, 384] per core
UW = KU * 4 * Z                        # 1536 unit columns
NF = 2 * FEM_F                         # 768 net+fem columns
LW = UW + NF                           # 2304 total columns
DREG = KU * 3 * 31                     # 1116 reg diff cols
ASPL = 672                             # reg diff cols squared on ACT

_PROGRAM = None
_HOOK_PATCHED = False
# Bump whenever the BIR post-edit logic changes: the neuron compile cache
# keys on the HLO (which embeds the *unpatched* BIR), so a patch-logic change
# must perturb the program to force a recompile.
_BIR_REV = 23


def _strip_pe(bir_json):
    """Delete every PE-engine instruction (register init + branches only --
    the kernel does no matmul).  With no PE instructions the NEFF carries no
    PE program, so the runtime entry rendezvous doesn't wait ~2.5us for the
    PE array init."""
    import json

    j = json.loads(bir_json)
    for f in j.get("functions", []):
        for bb in f.get("blocks", []):
            kept = []
            for i in bb.get("instructions", []):
                if i.get("engine") == "PE":
                    op = i.get("opcode")
                    assert op in ("RegisterMove", "UnconditionalBranch",
                                  "Drain", "EventSemaphore"), op
                else:
                    kept.append(i)
            bb["instructions"] = kept
    return json.dumps(j).encode()


def _fix_tail(bir_json):
    """Tail surgery.  (1) Strip the Tile entry barrier (block 0).  (2) In the
    end block: empty the multi-wait drain (walrus's >2-sync limit aside, its
    waits are covered by (3)), insert a Drain on the output DMA's engine that
    waits for the output DMAHW semaphore to reach 16 so NO engine reaches the
    runtime epilogue while the output write is in flight (the epilogue's
    semaphore-clear storm otherwise races the in-flight completion
    acknowledgments and stalls the semaphore bus for ~7us), keep the first
    barrier round with the Pool master counts dropped from 4 to 3 (PE is
    stripped), and delete the semaphore range-clear + second barrier round
    (the runtime epilogue re-zeroes every semaphore anyway)."""
    import json

    j = json.loads(bir_json)
    for f in j.get("functions", []):
        blocks = f.get("blocks", [])
        if not blocks:
            continue
        blocks[0]["instructions"] = [
            i for i in blocks[0].get("instructions", [])
            if i.get("opcode") not in ("Drain", "EventSemaphore")
        ]
        # Locate the output DMA (writes the "out" DRAM param).
        out_upd = out_eng = None
        for bb in blocks:
            for i in bb.get("instructions", []):
                if (i.get("opcode") == "DMACopy"
                        and (i.get("outs") or [{}])[0].get("memref") == "out"):
                    out_upd = (i["sync_info"]["on_update"] or [None])[0]
                    out_eng = i.get("engine")
        assert out_upd is not None and out_eng is not None
        bl = blocks[-1]
        insts = bl.get("instructions", [])
        # Drop the range-clear and the second barrier round after it.
        isa_idx = next((n for n, i in enumerate(insts)
                        if i.get("opcode") == "ISA"), len(insts))
        insts = insts[:isa_idx]
        for i in insts:
            si = i.get("sync_info") or {}
            if i.get("opcode") == "Drain" and len(si.get("on_wait") or []) > 1:
                si["on_wait"] = []
            if (i.get("opcode") == "EventSemaphore"
                    and i.get("engine") == "Pool"):
                for part in ("on_wait", "on_update"):
                    for e in si.get(part) or []:
                        if e.get("wait_value") == 4:
                            e["wait_value"] = 3
                        if e.get("update_value") == 4:
                            e["update_value"] = 3
        hold = {
            "debug": 1,
            "engine": out_eng,
            "ins": [],
            "name": "I-holdout",
            "opcode": "Drain",
            "outs": [],
            "sync_info": {
                "on_update": [],
                "on_wait": [{
                    "ant_name": out_upd["ant_name"],
                    "id": out_upd["id"],
                    "sync_type": "semaphore",
                    "wait_mode": "sem-ge-imm",
                    "wait_value": 16,
                }],
            },
        }
        bl["instructions"] = [hold] + insts
    return json.dumps(j).encode()


def _hoist_input_dmas(bir_json, input_names=("ld_a", "ld_b")):
    """Move the input-load DMA triggers to the head of the first block so the
    HBM->SBUF transfers start as soon as the runtime preamble ends, ahead of
    the register-init instructions.  The triggers have no waits and their
    DMAHW semaphore updates don't interact with anything earlier, so
    ordering stays sound."""
    import json

    j = json.loads(bir_json)
    for f in j.get("functions", []):
        blocks = f.get("blocks", [])
        if not blocks:
            continue
        existing = {i.get("name") for bb in blocks for i in bb.get("instructions", [])}
        hoisted = []
        for bb in blocks:
            insts = bb.get("instructions", [])
            keep = []
            for i in insts:
                ins0 = (i.get("ins") or [{}])[0]
                if (i.get("opcode") == "DMACopy"
                        and not (i.get("sync_info") or {}).get("on_wait")
                        and ins0.get("memref") in input_names):
                    hoisted.append(i)
                else:
                    keep.append(i)
            bb["instructions"] = keep
        # Renumber so they sort before everything even if the backend orders
        # by instruction id rather than list position.
        for n, i in enumerate(hoisted):
            name = f"I-{n}"
            while name in existing:
                name += "h"
            existing.add(name)
            i["name"] = name
            i["debug"] = 1
        blocks[0]["instructions"] = hoisted + blocks[0]["instructions"]
    return json.dumps(j).encode()


def _patch_compile_hook():
    global _HOOK_PATCHED
    if _HOOK_PATCHED:
        return
    import concourse.bass2jax as b2j

    orig = b2j.compile_bir_kernel

    def patched(bir_json, tmpdir, neff_name="file.neff"):
        return orig(_hoist_input_dmas(_fix_tail(
            _strip_pe(bir_json))), tmpdir, neff_name=neff_name)

    b2j.compile_bir_kernel = patched
    _HOOK_PATCHED = True


def _build_program():
    import concourse.bass as bass
    import concourse.mybir as mybir
    from concourse import tile
    from contextlib import ExitStack

    f32 = mybir.dt.float32
    bf16 = mybir.dt.bfloat16
    SUB = mybir.AluOpType.subtract
    MULT = mybir.AluOpType.mult
    ADD = mybir.AluOpType.add
    SQUARE = mybir.ActivationFunctionType.Square

    nc = bass.Bass()
    nc.dram_tensor(f"patchrev{_BIR_REV}", [1, 1], f32)
    ld_a = nc.declare_dram_parameter("ld_a", [128, UW], bf16, isOutput=False)
    ld_b = nc.declare_dram_parameter("ld_b", [128, NF], bf16, isOutput=False)
    out = nc.declare_dram_parameter("out", [1, 3], f32, isOutput=True)

    with tile.TileContext(nc) as tc, ExitStack() as ctx:
        pool = ctx.enter_context(tc.tile_pool(name="main", bufs=1))

        t_l = pool.tile([128, LW], bf16)
        nc.scalar.dma_start(out=t_l[:, 0:UW], in_=ld_a[:, :])
        nc.sync.dma_start(out=t_l[:, UW:LW], in_=ld_b[:, :])

        t_u = t_l[:, 0:UW].rearrange("p (k r z) -> p k r z", k=KU, r=4)
        t_d = pool.tile([128, DREG + FEM_F], bf16)
        t_sq = pool.tile([128, DREG], bf16)
        t_sqf = pool.tile([128, FEM_F], bf16)
        t_acc = pool.tile([128, 3], f32)
        t_out = pool.tile([1, 3], f32)
        t_sq2 = pool.tile([128, DREG - ASPL], bf16)

        d_reg = t_d[:, 0:DREG].rearrange("p (k r z) -> p k r z", k=KU, r=3)
        d_f = t_d[:, DREG:DREG + FEM_F]

        # Vector: fem subtract + fused square-accumulate, then the single
        # reg subtract (rows 1:4 minus broadcast row 0).
        nc.vector.tensor_tensor(
            out=d_f, in0=t_l[:, UW:UW + FEM_F],
            in1=t_l[:, UW + FEM_F:LW], op=SUB)
        nc.vector.scalar_tensor_tensor(
            out=t_sqf[:], in0=d_f, scalar=1.0, in1=d_f,
            op0=MULT, op1=MULT, accum_out=t_acc[:, 0:1])
        nc.vector.tensor_tensor(
            out=d_reg, in0=t_u[:, :, 1:4, 0:31],
            in1=t_u[:, :, 0:1, 0:31].broadcast_to([128, KU, 3, 31]), op=SUB)

        # Squares split across engines: ACT takes the first ASPL reg diff
        # columns, Vector the rest, each into its own accumulator column.
        nc.scalar.activation(
            out=t_sq[:, 0:ASPL], in_=t_d[:, 0:ASPL], func=SQUARE,
            accum_out=t_acc[:, 1:2])
        nc.vector.scalar_tensor_tensor(
            out=t_sq2[:], in0=t_d[:, ASPL:DREG], scalar=1.0,
            in1=t_d[:, ASPL:DREG], op0=MULT, op1=MULT,
            accum_out=t_acc[:, 2:3])

        # GpSimd folds the 128 per-partition partials to one row each
        # (one wait per op: column 0/2 are Vector's, column 1 is ACT's).
        nc.gpsimd.tensor_reduce(
            out=t_out[0:1, 0:2], in_=t_acc[:, 0:3:2], axis=AXC, op=ADD)
        nc.gpsimd.tensor_reduce(
            out=t_out[0:1, 2:3], in_=t_acc[:, 1:2], axis=AXC, op=ADD)

        # Single-descriptor output DMA on the ACT ring (waits GpSimd).
        nc.scalar.dma_start(out=out[:, :], in_=t_out[:])

    return nc


def _shard_inputs(network_mesh, fem_mesh, pred):
    import ml_dtypes
    bf16 = ml_dtypes.bfloat16
    predf = np.asarray(pred, dtype=np.float32).reshape(B * C, X, Y, Z)
    pad = N_CORES * PLANES_PC
    base_p = np.zeros((pad, Y, Z), np.float32)
    nxt_p = np.zeros((pad, Y, Z), np.float32)
    base_p[:REG_PLANES] = predf[:, : X - 1].reshape(REG_PLANES, Y, Z)
    nxt_p[:REG_PLANES] = predf[:, 1:].reshape(REG_PLANES, Y, Z)
    zsh = np.zeros((pad, Y - 1, Z), np.float32)
    zsh[:, :, : Z - 1] = base_p[:, : Y - 1, 1:]
    # [384, 31, 4, 32]: per (plane, y): base, y+1, x+1-plane, z+1 rows
    u_all = np.stack(
        [base_p[:, : Y - 1], base_p[:, 1:], nxt_p[:, : Y - 1], zsh], axis=2
    )
    netf = np.asarray(network_mesh, dtype=np.float32).reshape(N_CORES, FEM_P, FEM_F)
    femf = np.asarray(fem_mesh, dtype=np.float32).reshape(N_CORES, FEM_P, FEM_F)
    maps = []
    for c in range(N_CORES):
        uc = u_all[PLANES_PC * c : PLANES_PC * (c + 1)].reshape(UNITS_PC, 4, Z)
        up = np.zeros((128 * KU, 4, Z), np.float32)
        up[:UNITS_PC] = uc
        maps.append({
            "ld_a": up.reshape(128, UW).astype(bf16),
            "ld_b": np.ascontiguousarray(
                np.concatenate([netf[c], femf[c]], axis=1)).astype(bf16),
        })
    return maps


def run_sharded(network_mesh, fem_mesh, pred, trace=False):
    """Compile+run on 8 cores; returns (loss_scalar, BassKernelResults)."""
    global _PROGRAM
    from concourse.bass_utils import run_bass_kernel_spmd

    _patch_compile_hook()
    if _PROGRAM is None:
        _PROGRAM = _build_program()
    in_maps = _shard_inputs(network_mesh, fem_mesh, pred)
    res = run_bass_kernel_spmd(_PROGRAM, in_maps, list(range(N_CORES)), trace=trace)
    fem_sum = 0.0
    reg_sum = 0.0
    for c in range(N_CORES):
        o = np.asarray(res.results[c]["out"], dtype=np.float64).ravel()
        fem_sum += o[0]
        reg_sum += o[1] + o[2]
    loss = fem_sum / FEM_TOTAL + 0.1 * (reg_sum / (B * C))
    return np.asarray(loss, dtype=np.float32), res


def kernel(network_mesh, pc, fem_mesh, pred):
    loss, _ = run_sharded(network_mesh, fem_mesh, pred, trace=False)
    return loss
